# revision 29
# baseline (speedup 1.0000x reference)
"""Deformable convolution (DCNv1, 3x3, pad=1) on 8 Trainium2 NeuronCores.

Sharding: data-parallel over batch — one sample per core, weights replicated.

Per-core algorithm (v2 — prologue-free gather pipeline):
  1. Sampling positions (base grid + offset, minus 0.5) are host-staged in TWO
     layouts: pixel-major for the bilinear-weight math, and directly in the
     SWDGE wrap-16 per-queue layout for the gather indices.  The index math
     on DVE therefore writes the dma_gather index tile in place — no strided
     staging DMAs (the old kernel spent ~130us there).
  2. The -0.5 pre-bias makes the DVE's round-to-nearest int conversion act as
     floor(), removing the is_gt/subtract correction pass.
  3. One dma_gather descriptor per (tap, pixel) fetches the full 2x2 bilinear
     patch (512 fp16 values) from a row-pair-interleaved channels-last copy
     of the image in DRAM.  Calls rotate over the 4 SWDGE queues; descriptor
     generation on the Pool engine is the critical resource (~4.3ns/idx).
  4. Corner blending rides the PE's fp32 PSUM accumulation: per pixel block,
     4 matmuls against weighted-diagonal moving operands (dk) transpose the
     gathered patch to channel-major im2col columns while applying the
     bilinear corner weights.
  5. Conv = 9 accumulated fp16 matmuls into fp32 PSUM; bias on evacuation.

Numerics: gather/blend/cols/weights in fp16, PSUM accumulation fp32.
"""
from contextlib import ExitStack

import numpy as np

import concourse.bass as bass
import concourse.bacc as bacc
import concourse.tile as tile
from concourse import mybir
from concourse.bass import AP
from concourse import library_config
from concourse.bass_utils import run_bass_kernel_spmd

F32 = mybir.dt.float32
F16 = mybir.dt.float16
I32 = mybir.dt.int32
I16 = mybir.dt.int16

KH = KW = 3
K = 9
H = W = 64
HW = H * W
C = 128
O = 128
PAD_PX = 65
NV = 4352
TOT_PX = 4480
GELEM = 512          # one 2x2 patch: [x00|x10|x01|x11], fp16
GSTEP = 256          # slot stride (one pixel-row-pair slot)
NB = 32
CHUNKS = 2
NBC = NB // CHUNKS   # 16 blocks/chunk
PXC = HW // CHUNKS   # 2048 px/chunk
NQ = 4
CLQ = 10                 # call slots per queue (8x1024 + 2x512)

# corner order matches the gathered patch layout: slot ci = dx*2 + dy
CORNERS = ((0, 0), (1, 0), (0, 1), (1, 1))  # (dy, dx) for ci = 0..3


def _build_calls():
    """Gather call table: (queue, cl, ch, k, b0, nblk).

    Taps (0,0)..(1,6) use two 1024-idx calls each (half-tap granularity);
    the last two taps are split 4x512 so the end-of-stream DMA drain is
    half as deep.  Every queue gets exactly 9216 indices.
    """
    calls = []
    for g in range(32):
        ch, r = divmod(g, K * 2)
        k, half = divmod(r, 2)
        calls.append((g % NQ, g // NQ, ch, k, half * 8, 8))
    for t in range(8):
        calls.append((t % NQ, 8 + t // NQ, 1, 7 + t // 4, (t % 4) * 4, 4))
    return calls


_CALLS = _build_calls()

# ---- host-side index LUT for the wrap-16 per-queue position layout -------
# Descriptor j of a call lands at gk partition j%128, block b0 + j//128; its
# index is read from wrap position (j%16, j//16) of the queue's partitions.
_LUT_K = np.zeros((128, CLQ, 64), np.int64)
_LUT_PX = np.zeros((128, CLQ, 64), np.int64)
_LUT_V = np.zeros((128, CLQ, 64), bool)
for _q, _cl, _ch, _k, _b0, _nb in _CALLS:
    _s = np.arange(_nb * 8)
    for _h in (0, 16):
        for _w in range(16):
            _p = 32 * _q + _h + _w
            _j = _s * 16 + _w
            _px = _ch * 2048 + (_b0 + _j // 128) * 128 + (_j % 128)
            _LUT_K[_p, _cl, _s] = _k
            _LUT_PX[_p, _cl, _s] = _px
            _LUT_V[_p, _cl, _s] = True


def _prep_core_inputs(x_b, offset_b, weight, bias) -> dict:
    xclb = np.zeros((TOT_PX + W, C), np.float16)
    xclb[PAD_PX:PAD_PX + HW] = x_b.reshape(C, HW).T.astype(np.float16)
    xcl = np.zeros((TOT_PX, 2 * C), np.float16)
    xcl[:, :C] = xclb[:TOT_PX]
    xcl[:, C:] = xclb[W:TOT_PX + W]

    # positions = base grid + offset - 0.5 (pre-biased so round == floor)
    off = offset_b.reshape(K, 2, HW).astype(np.float32)
    p = np.arange(HW)
    py = (p // W).astype(np.float32)
    px = (p % W).astype(np.float32)
    base = np.empty((K, 2, HW), np.float32)
    for ki in range(KH):
        for kj in range(KW):
            k = ki * KW + kj
            base[k, 0] = py + ki
            base[k, 1] = px + kj
    pos = base + off - 0.5

    pos_w4 = np.ascontiguousarray(
        pos.reshape(K, 2, NB, 128).transpose(3, 0, 1, 2))
    pos_idx = np.stack([np.where(_LUT_V, pos[_LUT_K, 0, _LUT_PX], 0.5),
                        np.where(_LUT_V, pos[_LUT_K, 1, _LUT_PX], 0.5)],
                       axis=-1)
    pos_idx = np.ascontiguousarray(pos_idx, np.float32)

    wts = np.ascontiguousarray(
        weight.reshape(O, C, K).transpose(2, 1, 0)).astype(np.float16)
    return {
        "xcl": xcl,
        "pos_idx": pos_idx,
        "pos_w4": pos_w4,
        "wts": wts,
        "bias_in": bias.reshape(O, 1).astype(np.float32),
        "ident_in": np.eye(128, dtype=np.float16),
    }


def _dcn_core_kernel(tc, outs, ins):
    nc = tc.nc
    out_d = outs["out"]
    A = mybir.AluOpType

    with ExitStack() as ctx:
        consts = ctx.enter_context(tc.tile_pool(name="consts", bufs=1))
        idxp = ctx.enter_context(tc.tile_pool(name="idx", bufs=1))
        gath = ctx.enter_context(tc.tile_pool(name="gath", bufs=4))
        pmp = ctx.enter_context(tc.tile_pool(name="pm", bufs=2))
        wrp = ctx.enter_context(tc.tile_pool(name="wrp", bufs=2))
        colp = ctx.enter_context(tc.tile_pool(name="col", bufs=2))
        outp = ctx.enter_context(tc.tile_pool(name="outsb", bufs=2))
        psums = ctx.enter_context(tc.tile_pool(name="psums", bufs=4, space="PSUM"))
        psumc = ctx.enter_context(tc.tile_pool(name="psumc", bufs=1, space="PSUM"))

        pos_idx = consts.tile([128, CLQ, 64, 2], F32)
        pos_w4 = consts.tile([128, K, 2, NB], F32)
        # first-call slice lands first so the index math (and the first
        # gather) starts as early as possible
        nc.sync.dma_start(out=pos_idx[:, 0:1], in_=ins["pos_idx"][:, 0:1])
        nc.sync.dma_start(out=pos_idx[:, 1:], in_=ins["pos_idx"][:, 1:])
        nc.scalar.dma_start(out=pos_w4, in_=ins["pos_w4"])
        # one strided DMA: dram [K, C, O] -> sbuf [c-part, K, O]
        wts = consts.tile([128, K, O], F16)
        wsrc = ins["wts"]
        w_ap = bass.AP(tensor=wsrc.tensor, offset=0,
                       ap=[[O, 128], [C * O, K], [1, O]])
        nc.scalar.dma_start(out=wts, in_=w_ap)
        bias_sb = consts.tile([128, 1], F32)
        nc.sync.dma_start(out=bias_sb, in_=ins["bias_in"])
        ident = consts.tile([128, 128], F16)
        nc.sync.dma_start(out=ident, in_=ins["ident_in"])
        nc.gpsimd.load_library(library_config.mlp)

        # ---- gather indices, computed straight into SWDGE wrap-16 layout.
        # Two passes: cl=0 (first 4 calls) first so gathers start early.
        xview = AP(tensor=ins["xcl"].tensor, offset=0,
                   ap=[[GSTEP, NV], [1, GELEM]])

        idxw_parts = []
        early_gk = {}
        for pi, sl in enumerate((slice(0, 1), slice(1, CLQ))):
            n = sl.stop - sl.start
            cpos = idxp.tile([128, n, 64, 2], F32, name=f"cpos{pi}")
            fi = idxp.tile([128, n, 64, 2], I32, name=f"fi{pi}")
            ff = idxp.tile([128, n, 64, 2], F32, name=f"ff{pi}")
            gf = idxp.tile([128, n, 64], F32, name=f"gf{pi}")
            idxw_p = idxp.tile([128, n, 64], I16, name=f"idxw{pi}")
            nc.vector.tensor_scalar(out=cpos, in0=pos_idx[:, sl],
                                    scalar1=-0.5, scalar2=64.5,
                                    op0=A.max, op1=A.min)
            nc.vector.tensor_copy(out=fi, in_=cpos)
            nc.vector.tensor_copy(out=ff, in_=fi)
            nc.vector.scalar_tensor_tensor(out=gf, in0=ff[:, :, :, 0],
                                           scalar=64.0, in1=ff[:, :, :, 1],
                                           op0=A.mult, op1=A.add)
            nc.vector.tensor_copy(out=idxw_p, in_=gf)
            idxw_parts.append(idxw_p)

        def idx_ap(cl, nslot):
            if cl == 0:
                return idxw_parts[0][:, 0, :nslot]
            return idxw_parts[1][:, cl - 1, :nslot]

        tap_calls = {}
        for c in _CALLS:
            tap_calls.setdefault((c[2], c[3]), []).append(c)

        # ---- bilinear corner weights (pixel-major; validity masks folded in)
        cw = idxp.tile([128, K, 2, NB], F32)
        nc.vector.tensor_scalar(out=cw, in0=pos_w4, scalar1=-0.5, scalar2=64.5,
                                op0=A.max, op1=A.min)
        fiw = idxp.tile([128, K, 2, NB], I32)
        nc.vector.tensor_copy(out=fiw, in_=cw)
        fw = idxp.tile([128, K, 2, NB], F32)
        nc.vector.tensor_copy(out=fw, in_=fiw)
        frac = idxp.tile([128, K, 2, NB], F32)
        nc.vector.scalar_tensor_tensor(out=frac, in0=cw, scalar=0.5, in1=fw,
                                       op0=A.add, op1=A.subtract)
        va = idxp.tile([128, K, 2, NB], F32)
        vb = idxp.tile([128, K, 2, NB], F32)
        nc.vector.tensor_scalar(out=va, in0=fw, scalar1=1.0, scalar2=None,
                                op0=A.is_ge)
        nc.vector.tensor_scalar(out=vb, in0=fw, scalar1=64.0, scalar2=None,
                                op0=A.is_le)
        nc.vector.tensor_tensor(out=va, in0=va, in1=vb, op=A.mult)
        nc.vector.tensor_scalar(out=vb, in0=fw, scalar1=63.0, scalar2=None,
                                op0=A.is_le)
        w0 = idxp.tile([128, K, 2, NB], F32)
        w1 = idxp.tile([128, K, 2, NB], F32)
        nc.vector.tensor_scalar(out=w0, in0=frac, scalar1=-1.0, scalar2=1.0,
                                op0=A.mult, op1=A.add)
        nc.vector.tensor_tensor(out=w0, in0=w0, in1=va, op=A.mult)
        nc.vector.tensor_tensor(out=w1, in0=frac, in1=vb, op=A.mult)
        w4 = idxp.tile([128, K, 4, NB], F16)
        wy = (w0, w1)
        wx = (w0, w1)
        for ci, (dy, dx) in enumerate(CORNERS):
            nc.vector.tensor_tensor(
                out=w4[:, :, ci, :], in0=wy[dy][:, :, 0, :], in1=wx[dx][:, :, 1, :],
                op=A.mult)

        for ch in range(CHUNKS):
            # one 1-bank PSUM tile per 512-column accumulation group —
            # chunk 2's group m only waits for chunk 1's group m to evacuate
            conv_ms = [psumc.tile([128, 512], F32, space="PSUM",
                                  name=f"convps{m}") for m in range(PXC // 512)]
            bs = ch * NBC
            for k in range(K):
                gk = early_gk.pop((ch, k), None)
                if gk is None:
                    gk = gath.tile([128, NBC, GELEM], F16, name="gk")
                    for q, cl, _, _, b0, nb in tap_calls[(ch, k)]:
                        nc.gpsimd.dma_gather(
                            out_ap=gk[:, b0:b0 + nb, :],
                            in_ap=xview,
                            idxs_ap=idx_ap(cl, nb * 8),
                            num_idxs=nb * 128,
                            num_idxs_reg=nb * 128,
                            elem_size=GELEM,
                            elem_step=GSTEP,
                            queue_num=q,
                        )
                # weighted-diagonal moving operands: Dk[q, ci, b, j] =
                # ident[q, j] * w4[q, k, ci, bs+b]; the corner SUM rides the
                # PE's fp32 PSUM accumulation.  For a third of the taps, ACT
                # materialises the broadcast weights so the DVE multiply is a
                # stride-1 fp16 tensor_tensor (2x DVE perf mode) — balances
                # the diag-build load between ACT and DVE.
                dk = pmp.tile([128, 4, NBC, C], F16)
                i0 = ident[:, :]
                ident_b = bass.AP(tensor=i0.tensor, offset=i0.offset,
                                  ap=[i0.ap[0], [0, 4], [0, NBC], [1, C]])
                wv = w4[:, k, :, bs:bs + NBC]
                w_b = bass.AP(tensor=wv.tensor, offset=wv.offset,
                              ap=[wv.ap[0], wv.ap[1], wv.ap[2], [0, C]])
                if (ch * K + k) % 3 == 0:
                    w4rep = wrp.tile([128, 4, NBC, C], F16)
                    nc.scalar.copy(out=w4rep, in_=w_b)
                    nc.vector.tensor_tensor(out=dk[:, :, :, :], in0=ident_b,
                                            in1=w4rep, op=A.mult)
                else:
                    nc.vector.tensor_tensor(out=dk[:, :, :, :], in0=ident_b,
                                            in1=w_b, op=A.mult)
                # per pixel block: psum[c, j] += sum_ci gk_ci.T @ diag(w_ci)
                colk = colp.tile([128, PXC], F16)
                for bg in range(NBC // 4):
                    pst = psums.tile([128, 512], F32, space="PSUM")
                    for j in range(4):
                        b = bg * 4 + j
                        for ci in range(4):
                            nc.tensor.matmul(
                                out=pst[:, j * 128:(j + 1) * 128],
                                lhsT=gk[:, b, ci * C:(ci + 1) * C],
                                rhs=dk[:, ci, b, :],
                                start=(ci == 0), stop=(ci == 3))
                    nc.scalar.copy(out=colk[:, bg * 512:(bg + 1) * 512], in_=pst)
                for m in range(PXC // 512):
                    nc.tensor.matmul(
                        out=conv_ms[m][:, :],
                        lhsT=wts[:, k, :],
                        rhs=colk[:, m * 512:(m + 1) * 512],
                        start=(k == 0), stop=(k == K - 1))
            # evacuate per 512-column accumulation group so the tail pipelines
            # with the final conv matmuls
            out_sb = outp.tile([128, PXC], F32)
            for m in range(PXC // 512):
                nc.scalar.activation(out=out_sb[:, m * 512:(m + 1) * 512],
                                     in_=conv_ms[m][:, :],
                                     func=mybir.ActivationFunctionType.Identity,
                                     bias=bias_sb[:, :], scale=1.0)
                nc.sync.dma_start(
                    out=out_d[:, ch * PXC + m * 512:ch * PXC + (m + 1) * 512],
                    in_=out_sb[:, m * 512:(m + 1) * 512])


_IN_SPECS = {
    "xcl": ((TOT_PX, 2 * C), np.float16),
    "pos_idx": ((128, CLQ, 64, 2), np.float32),
    "pos_w4": ((128, K, 2, NB), np.float32),
    "wts": ((K, C, O), np.float16),
    "bias_in": ((O, 1), np.float32),
    "ident_in": ((128, 128), np.float16),
}

_prog_cache = {}


def _build_program():
    if "nc" in _prog_cache:
        return _prog_cache["nc"]
    nc = bacc.Bacc("TRN2", target_bir_lowering=False, debug=False,
                   num_swdge_queues=NQ)
    ins = {}
    for name, (shape, dtype) in _IN_SPECS.items():
        ins[name] = nc.dram_tensor(
            name, list(shape), mybir.dt.from_np(np.dtype(dtype)),
            kind="ExternalInput").ap()
    outs = {"out": nc.dram_tensor("out", [O, HW], F32,
                                  kind="ExternalOutput").ap()}
    with tile.TileContext(nc) as tc:
        _dcn_core_kernel(tc, outs, ins)
    nc.compile()
    _prog_cache["nc"] = nc
    return nc


def run_dcn(x, offset, weight, bias, trace=False):
    x = np.ascontiguousarray(x, dtype=np.float32)
    offset = np.ascontiguousarray(offset, dtype=np.float32)
    weight = np.ascontiguousarray(weight, dtype=np.float32)
    bias = np.ascontiguousarray(bias, dtype=np.float32)
    B = x.shape[0]
    in_maps = [_prep_core_inputs(x[b], offset[b], weight, bias)
               for b in range(B)]
    nc = _build_program()
    res = run_bass_kernel_spmd(nc, in_maps, core_ids=list(range(B)), trace=trace)
    out = np.stack([r["out"] for r in res.results]).reshape(B, O, H, W)
    return out, res


def kernel(x, offset, weight, bias):
    out, _ = run_dcn(x, offset, weight, bias)
    return out.astype(np.float32)


# revision 36
# speedup vs baseline: 1.0291x; 1.0291x over previous
"""Deformable convolution (DCNv1, 3x3, pad=1) on 8 Trainium2 NeuronCores.

Sharding: data-parallel over batch — one sample per core, weights replicated.

Per-core algorithm (prologue-free gather pipeline):
  1. Sampling positions (base grid + offset, minus 0.5) are host-staged in TWO
     layouts: pixel-major for the bilinear-weight math, and directly in the
     SWDGE wrap-16 per-queue layout for the gather indices.  The index math
     on DVE therefore writes the dma_gather index tile in place — no strided
     staging DMAs (an earlier version spent ~130us there).
  2. The -0.5 pre-bias makes the DVE's round-to-nearest int conversion act as
     floor(), removing the is_gt/subtract correction pass.
  3. One dma_gather descriptor per (tap, pixel) fetches the full 2x2 bilinear
     patch (512 fp16 values) from a row-pair-interleaved channels-last copy
     of the image in DRAM.  Calls rotate over the 4 SWDGE queues; descriptor
     generation runs concurrently on the 4 Q7 cpu pairs (~17us per 1024-idx
     call per pair) and is the structural critical resource (~153us/core).
  4. Corner blending rides the PE's fp32 PSUM accumulation: per pixel block,
     4 matmuls against weighted-diagonal moving operands (dk) transpose the
     gathered patch to channel-major im2col columns while applying the
     bilinear corner weights.  For a third of the taps ACT materialises the
     broadcast weights so the DVE diag-build is a stride-1 fp16
     tensor_tensor that hits the 2x DVE perf mode.
  5. Conv = 9 accumulated fp16 matmuls into one 1-bank PSUM tile per
     512-column group (fine-grained chunk handoff); bias on evacuation,
     which is split per group so the tail pipelines with the last matmuls.

Numerics: gather/blend/cols/weights in fp16, PSUM accumulation fp32.
Measured: 186us/core (baseline 317us), rel err 4.5e-4.
"""
from contextlib import ExitStack

import numpy as np

import concourse.bass as bass
import concourse.bacc as bacc
import concourse.tile as tile
from concourse import mybir
from concourse.bass import AP
from concourse import library_config
from concourse.bass_utils import run_bass_kernel_spmd

F32 = mybir.dt.float32
F16 = mybir.dt.float16
I32 = mybir.dt.int32
I16 = mybir.dt.int16

KH = KW = 3
K = 9
H = W = 64
HW = H * W
C = 128
O = 128
PAD_PX = 65
NV = 4352
TOT_PX = 4480
GELEM = 512          # one 2x2 patch: [x00|x10|x01|x11], fp16
GSTEP = 256          # slot stride (one pixel-row-pair slot)
NB = 32
CHUNKS = 2
NBC = NB // CHUNKS   # 16 blocks/chunk
PXC = HW // CHUNKS   # 2048 px/chunk
NCALL = CHUNKS * K * 2   # 36 gather calls, 1024 idx each
NQ = 4
CLQ = NCALL // NQ        # 9 calls per queue

# corner order matches the gathered patch layout: slot ci = dx*2 + dy
CORNERS = ((0, 0), (1, 0), (0, 1), (1, 1))  # (dy, dx) for ci = 0..3

# ---- host-side index LUT for the wrap-16 per-queue position layout -------
# Call g = ch*18 + k*2 + half runs on queue g%4 and is that queue's (g//4)-th
# call.  Descriptor j of a call lands at gk partition j%128, block j//128; its
# index is read from wrap position (j%16, j//16) of the queue's partitions.
_PP, _CL, _S = np.meshgrid(
    np.arange(128), np.arange(CLQ), np.arange(64), indexing="ij")
_G = _CL * NQ + (_PP // 32)
_CHg, _Rg = _G // (K * 2), _G % (K * 2)
_Kg, _HALFg = _Rg // 2, _Rg % 2
_J = _S * 16 + (_PP % 16)
_PXLg = _CHg * 2048 + _HALFg * 1024 + (_J // 128) * 128 + (_J % 128)


def _prep_core_inputs(x_b, offset_b, weight, bias) -> dict:
    xclb = np.zeros((TOT_PX + W, C), np.float16)
    xclb[PAD_PX:PAD_PX + HW] = x_b.reshape(C, HW).T.astype(np.float16)
    xcl = np.zeros((TOT_PX, 2 * C), np.float16)
    xcl[:, :C] = xclb[:TOT_PX]
    xcl[:, C:] = xclb[W:TOT_PX + W]

    # positions = base grid + offset - 0.5 (pre-biased so round == floor)
    off = offset_b.reshape(K, 2, HW).astype(np.float32)
    p = np.arange(HW)
    py = (p // W).astype(np.float32)
    px = (p % W).astype(np.float32)
    base = np.empty((K, 2, HW), np.float32)
    for ki in range(KH):
        for kj in range(KW):
            k = ki * KW + kj
            base[k, 0] = py + ki
            base[k, 1] = px + kj
    pos = base + off - 0.5

    pos_w4 = np.ascontiguousarray(
        pos.reshape(K, 2, NB, 128).transpose(3, 0, 1, 2))
    pos_idx = np.stack([pos[_Kg, 0, _PXLg], pos[_Kg, 1, _PXLg]], axis=-1)
    pos_idx = np.ascontiguousarray(pos_idx, np.float32)

    wts = np.ascontiguousarray(
        weight.reshape(O, C, K).transpose(2, 1, 0)).astype(np.float16)
    return {
        "xcl": xcl,
        "pos_idx": pos_idx,
        "pos_w4": pos_w4,
        "wts": wts,
        "bias_in": bias.reshape(O, 1).astype(np.float32),
        "ident_in": np.eye(128, dtype=np.float16),
    }


def _dcn_core_kernel(tc, outs, ins):
    nc = tc.nc
    out_d = outs["out"]
    A = mybir.AluOpType

    with ExitStack() as ctx:
        consts = ctx.enter_context(tc.tile_pool(name="consts", bufs=1))
        idxp = ctx.enter_context(tc.tile_pool(name="idx", bufs=1))
        gath = ctx.enter_context(tc.tile_pool(name="gath", bufs=4))
        pmp = ctx.enter_context(tc.tile_pool(name="pm", bufs=2))
        wrp = ctx.enter_context(tc.tile_pool(name="wrp", bufs=2))
        colp = ctx.enter_context(tc.tile_pool(name="col", bufs=2))
        outp = ctx.enter_context(tc.tile_pool(name="outsb", bufs=2))
        psums = ctx.enter_context(tc.tile_pool(name="psums", bufs=4, space="PSUM"))
        psumc = ctx.enter_context(tc.tile_pool(name="psumc", bufs=1, space="PSUM"))

        pos_idx = consts.tile([128, CLQ, 64, 2], F32)
        pos_w4 = consts.tile([128, K, 2, NB], F32)
        # first-call slice lands first so the index math (and the first
        # gather) starts as early as possible
        nc.sync.dma_start(out=pos_idx[:, 0:1], in_=ins["pos_idx"][:, 0:1])
        nc.sync.dma_start(out=pos_idx[:, 1:], in_=ins["pos_idx"][:, 1:])
        nc.scalar.dma_start(out=pos_w4, in_=ins["pos_w4"])
        # one strided DMA: dram [K, C, O] -> sbuf [c-part, K, O]
        wts = consts.tile([128, K, O], F16)
        wsrc = ins["wts"]
        w_ap = bass.AP(tensor=wsrc.tensor, offset=0,
                       ap=[[O, 128], [C * O, K], [1, O]])
        nc.scalar.dma_start(out=wts, in_=w_ap)
        bias_sb = consts.tile([128, 1], F32)
        nc.sync.dma_start(out=bias_sb, in_=ins["bias_in"])
        ident = consts.tile([128, 128], F16)
        nc.sync.dma_start(out=ident, in_=ins["ident_in"])
        nc.gpsimd.load_library(library_config.mlp)

        # ---- gather indices, computed straight into SWDGE wrap-16 layout.
        # Two passes: cl=0 (first 4 calls) first so gathers start early.
        xview = AP(tensor=ins["xcl"].tensor, offset=0,
                   ap=[[GSTEP, NV], [1, GELEM]])

        idxw_parts = []
        for pi, sl in enumerate((slice(0, 1), slice(1, CLQ))):
            n = sl.stop - sl.start
            cpos = idxp.tile([128, n, 64, 2], F32, name=f"cpos{pi}")
            fi = idxp.tile([128, n, 64, 2], I32, name=f"fi{pi}")
            ff = idxp.tile([128, n, 64, 2], F32, name=f"ff{pi}")
            gf = idxp.tile([128, n, 64], F32, name=f"gf{pi}")
            idxw_p = idxp.tile([128, n, 64], I16, name=f"idxw{pi}")
            nc.vector.tensor_scalar(out=cpos, in0=pos_idx[:, sl],
                                    scalar1=-0.5, scalar2=64.5,
                                    op0=A.max, op1=A.min)
            nc.vector.tensor_copy(out=fi, in_=cpos)
            nc.vector.tensor_copy(out=ff, in_=fi)
            nc.vector.scalar_tensor_tensor(out=gf, in0=ff[:, :, :, 0],
                                           scalar=64.0, in1=ff[:, :, :, 1],
                                           op0=A.mult, op1=A.add)
            nc.vector.tensor_copy(out=idxw_p, in_=gf)
            idxw_parts.append(idxw_p)

        def idxw_call(cl):
            return idxw_parts[0][:, 0, :] if cl == 0 else idxw_parts[1][:, cl - 1, :]

        # ---- bilinear corner weights (pixel-major; validity masks folded in)
        cw = idxp.tile([128, K, 2, NB], F32)
        nc.vector.tensor_scalar(out=cw, in0=pos_w4, scalar1=-0.5, scalar2=64.5,
                                op0=A.max, op1=A.min)
        fiw = idxp.tile([128, K, 2, NB], I32)
        nc.vector.tensor_copy(out=fiw, in_=cw)
        fw = idxp.tile([128, K, 2, NB], F32)
        nc.vector.tensor_copy(out=fw, in_=fiw)
        frac = idxp.tile([128, K, 2, NB], F32)
        nc.vector.scalar_tensor_tensor(out=frac, in0=cw, scalar=0.5, in1=fw,
                                       op0=A.add, op1=A.subtract)
        va = idxp.tile([128, K, 2, NB], F32)
        vb = idxp.tile([128, K, 2, NB], F32)
        nc.vector.tensor_scalar(out=va, in0=fw, scalar1=1.0, scalar2=None,
                                op0=A.is_ge)
        nc.vector.tensor_scalar(out=vb, in0=fw, scalar1=64.0, scalar2=None,
                                op0=A.is_le)
        nc.vector.tensor_tensor(out=va, in0=va, in1=vb, op=A.mult)
        nc.vector.tensor_scalar(out=vb, in0=fw, scalar1=63.0, scalar2=None,
                                op0=A.is_le)
        w0 = idxp.tile([128, K, 2, NB], F32)
        w1 = idxp.tile([128, K, 2, NB], F32)
        nc.vector.tensor_scalar(out=w0, in0=frac, scalar1=-1.0, scalar2=1.0,
                                op0=A.mult, op1=A.add)
        nc.vector.tensor_tensor(out=w0, in0=w0, in1=va, op=A.mult)
        nc.vector.tensor_tensor(out=w1, in0=frac, in1=vb, op=A.mult)
        w4 = idxp.tile([128, K, 4, NB], F16)
        wy = (w0, w1)
        wx = (w0, w1)
        for ci, (dy, dx) in enumerate(CORNERS):
            nc.vector.tensor_tensor(
                out=w4[:, :, ci, :], in0=wy[dy][:, :, 0, :], in1=wx[dx][:, :, 1, :],
                op=A.mult)

        qidx = 0
        for ch in range(CHUNKS):
            # one 1-bank PSUM tile per 512-column accumulation group —
            # chunk 2's group m only waits for chunk 1's group m to evacuate
            conv_ms = [psumc.tile([128, 512], F32, space="PSUM",
                                  name=f"convps{m}") for m in range(PXC // 512)]
            bs = ch * NBC
            for k in range(K):
                gk = gath.tile([128, NBC, GELEM], F16, name="gk")
                for s in range(2):
                    nc.gpsimd.dma_gather(
                        out_ap=gk[:, s * 8:(s + 1) * 8, :],
                        in_ap=xview,
                        idxs_ap=idxw_call(qidx // NQ),
                        num_idxs=1024,
                        num_idxs_reg=1024,
                        elem_size=GELEM,
                        elem_step=GSTEP,
                        queue_num=qidx % NQ,
                    )
                    qidx += 1
                # weighted-diagonal moving operands: Dk[q, ci, b, j] =
                # ident[q, j] * w4[q, k, ci, bs+b]; the corner SUM rides the
                # PE's fp32 PSUM accumulation.  For a third of the taps, ACT
                # materialises the broadcast weights so the DVE multiply is a
                # stride-1 fp16 tensor_tensor (2x DVE perf mode) — balances
                # the diag-build load between ACT and DVE.
                dk = pmp.tile([128, 4, NBC, C], F16)
                i0 = ident[:, :]
                ident_b = bass.AP(tensor=i0.tensor, offset=i0.offset,
                                  ap=[i0.ap[0], [0, 4], [0, NBC], [1, C]])
                wv = w4[:, k, :, bs:bs + NBC]
                w_b = bass.AP(tensor=wv.tensor, offset=wv.offset,
                              ap=[wv.ap[0], wv.ap[1], wv.ap[2], [0, C]])
                if (ch * K + k) % 3 == 0:
                    w4rep = wrp.tile([128, 4, NBC, C], F16)
                    nc.scalar.copy(out=w4rep, in_=w_b)
                    nc.vector.tensor_tensor(out=dk[:, :, :, :], in0=ident_b,
                                            in1=w4rep, op=A.mult)
                else:
                    nc.vector.tensor_tensor(out=dk[:, :, :, :], in0=ident_b,
                                            in1=w_b, op=A.mult)
                # per pixel block: psum[c, j] += sum_ci gk_ci.T @ diag(w_ci)
                colk = colp.tile([128, PXC], F16)
                for bg in range(NBC // 4):
                    pst = psums.tile([128, 512], F32, space="PSUM")
                    for j in range(4):
                        b = bg * 4 + j
                        for ci in range(4):
                            nc.tensor.matmul(
                                out=pst[:, j * 128:(j + 1) * 128],
                                lhsT=gk[:, b, ci * C:(ci + 1) * C],
                                rhs=dk[:, ci, b, :],
                                start=(ci == 0), stop=(ci == 3))
                    nc.scalar.copy(out=colk[:, bg * 512:(bg + 1) * 512], in_=pst)
                for m in range(PXC // 512):
                    nc.tensor.matmul(
                        out=conv_ms[m][:, :],
                        lhsT=wts[:, k, :],
                        rhs=colk[:, m * 512:(m + 1) * 512],
                        start=(k == 0), stop=(k == K - 1))
            # evacuate per 512-column accumulation group so the tail pipelines
            # with the final conv matmuls
            out_sb = outp.tile([128, PXC], F32)
            for m in range(PXC // 512):
                nc.scalar.activation(out=out_sb[:, m * 512:(m + 1) * 512],
                                     in_=conv_ms[m][:, :],
                                     func=mybir.ActivationFunctionType.Identity,
                                     bias=bias_sb[:, :], scale=1.0)
                nc.sync.dma_start(
                    out=out_d[:, ch * PXC + m * 512:ch * PXC + (m + 1) * 512],
                    in_=out_sb[:, m * 512:(m + 1) * 512])


_IN_SPECS = {
    "xcl": ((TOT_PX, 2 * C), np.float16),
    "pos_idx": ((128, CLQ, 64, 2), np.float32),
    "pos_w4": ((128, K, 2, NB), np.float32),
    "wts": ((K, C, O), np.float16),
    "bias_in": ((O, 1), np.float32),
    "ident_in": ((128, 128), np.float16),
}

_prog_cache = {}


def _build_program():
    if "nc" in _prog_cache:
        return _prog_cache["nc"]
    nc = bacc.Bacc("TRN2", target_bir_lowering=False, debug=False,
                   num_swdge_queues=NQ)
    ins = {}
    for name, (shape, dtype) in _IN_SPECS.items():
        ins[name] = nc.dram_tensor(
            name, list(shape), mybir.dt.from_np(np.dtype(dtype)),
            kind="ExternalInput").ap()
    outs = {"out": nc.dram_tensor("out", [O, HW], F32,
                                  kind="ExternalOutput").ap()}
    with tile.TileContext(nc) as tc:
        _dcn_core_kernel(tc, outs, ins)
    nc.compile()
    _prog_cache["nc"] = nc
    return nc


def run_dcn(x, offset, weight, bias, trace=False):
    x = np.ascontiguousarray(x, dtype=np.float32)
    offset = np.ascontiguousarray(offset, dtype=np.float32)
    weight = np.ascontiguousarray(weight, dtype=np.float32)
    bias = np.ascontiguousarray(bias, dtype=np.float32)
    B = x.shape[0]
    in_maps = [_prep_core_inputs(x[b], offset[b], weight, bias)
               for b in range(B)]
    nc = _build_program()
    res = run_bass_kernel_spmd(nc, in_maps, core_ids=list(range(B)), trace=trace)
    out = np.stack([r["out"] for r in res.results]).reshape(B, O, H, W)
    return out, res


def kernel(x, offset, weight, bias):
    out, _ = run_dcn(x, offset, weight, bias)
    return out.astype(np.float32)


# revision 38
# speedup vs baseline: 1.0737x; 1.0433x over previous
"""Deformable convolution (DCNv1, 3x3, pad=1) on 8 Trainium2 NeuronCores.

Sharding: data-parallel over batch — one sample per core, weights replicated.

Per-core algorithm (prologue-free gather pipeline):
  1. Sampling positions (base grid + offset, minus 0.5) are host-staged in TWO
     layouts: pixel-major for the bilinear-weight math, and directly in the
     SWDGE wrap-16 per-queue layout for the gather indices.  The index math
     on DVE therefore writes the dma_gather index tile in place — no strided
     staging DMAs (an earlier version spent ~130us there).
  2. The -0.5 pre-bias makes the DVE's round-to-nearest int conversion act as
     floor(), removing the is_gt/subtract correction pass.
  3. One dma_gather descriptor per (tap, pixel) fetches the full 2x2 bilinear
     patch (512 fp16 values) from a row-pair-interleaved channels-last copy
     of the image in DRAM.  Calls rotate over the 4 SWDGE queues; descriptor
     generation runs concurrently on the 4 Q7 cpu pairs (~17us per 1024-idx
     call per pair) and is the structural critical resource (~153us/core).
  4. Corner blending rides the PE's fp32 PSUM accumulation: per pixel block,
     4 matmuls against weighted-diagonal moving operands (dk) transpose the
     gathered patch to channel-major im2col columns while applying the
     bilinear corner weights.  For a third of the taps ACT materialises the
     broadcast weights so the DVE diag-build is a stride-1 fp16
     tensor_tensor that hits the 2x DVE perf mode.
  5. Conv = 9 accumulated fp16 matmuls into one 1-bank PSUM tile per
     512-column group (fine-grained chunk handoff); bias on evacuation,
     which is split per group so the tail pipelines with the last matmuls.

Numerics: gather/blend/cols/weights in fp16, PSUM accumulation fp32.
Measured: 186us/core (baseline 317us), rel err 4.5e-4.
"""
from contextlib import ExitStack

import numpy as np

import concourse.bass as bass
import concourse.bacc as bacc
import concourse.tile as tile
from concourse import mybir
from concourse.bass import AP
from concourse import library_config
from concourse.bass_utils import run_bass_kernel_spmd

F32 = mybir.dt.float32
F16 = mybir.dt.float16
I32 = mybir.dt.int32
I16 = mybir.dt.int16

KH = KW = 3
K = 9
H = W = 64
HW = H * W
C = 128
O = 128
PAD_PX = 65
NV = 4352
TOT_PX = 4480
GELEM = 512          # one 2x2 patch: [x00|x10|x01|x11], fp16
GSTEP = 256          # slot stride (one pixel-row-pair slot)
NB = 32
CHUNKS = 2
NBC = NB // CHUNKS   # 16 blocks/chunk
PXC = HW // CHUNKS   # 2048 px/chunk
NCALL = CHUNKS * K * 2   # 36 gather calls, 1024 idx each
NQ = 4
CLQ = NCALL // NQ        # 9 calls per queue

# corner order matches the gathered patch layout: slot ci = dx*2 + dy
CORNERS = ((0, 0), (1, 0), (0, 1), (1, 1))  # (dy, dx) for ci = 0..3

# ---- host-side index LUT for the wrap-16 per-queue position layout -------
# Call g = ch*18 + k*2 + half runs on queue g%4 and is that queue's (g//4)-th
# call.  Descriptor j of a call lands at gk partition j%128, block j//128; its
# index is read from wrap position (j%16, j//16) of the queue's partitions.
_PP, _CL, _S = np.meshgrid(
    np.arange(128), np.arange(CLQ), np.arange(64), indexing="ij")
_G = _CL * NQ + (_PP // 32)
_CHg, _Rg = _G // (K * 2), _G % (K * 2)
_Kg, _HALFg = _Rg // 2, _Rg % 2
_J = _S * 16 + (_PP % 16)
_PXLg = _CHg * 2048 + _HALFg * 1024 + (_J // 128) * 128 + (_J % 128)


def _prep_core_inputs(x_b, offset_b, weight, bias) -> dict:
    xclb = np.zeros((TOT_PX + W, C), np.float16)
    xclb[PAD_PX:PAD_PX + HW] = x_b.reshape(C, HW).T.astype(np.float16)
    xcl = np.zeros((TOT_PX, 2 * C), np.float16)
    xcl[:, :C] = xclb[:TOT_PX]
    xcl[:, C:] = xclb[W:TOT_PX + W]

    # positions = base grid + offset - 0.5 (pre-biased so round == floor)
    off = offset_b.reshape(K, 2, HW).astype(np.float32)
    p = np.arange(HW)
    py = (p // W).astype(np.float32)
    px = (p % W).astype(np.float32)
    base = np.empty((K, 2, HW), np.float32)
    for ki in range(KH):
        for kj in range(KW):
            k = ki * KW + kj
            base[k, 0] = py + ki
            base[k, 1] = px + kj
    pos = base + off - 0.5

    pos_w4 = np.ascontiguousarray(
        pos.reshape(K, 2, NB, 128).transpose(3, 0, 1, 2))
    pos_idx = np.stack([pos[_Kg, 0, _PXLg], pos[_Kg, 1, _PXLg]], axis=-1)
    pos_idx = np.ascontiguousarray(pos_idx, np.float32)

    wts = np.ascontiguousarray(
        weight.reshape(O, C, K).transpose(2, 1, 0)).astype(np.float16)
    return {
        "xcl": xcl,
        "pos_idx": pos_idx,
        "pos_w4": pos_w4,
        "wts": wts,
        "bias_in": bias.reshape(O, 1).astype(np.float32),
        "ident_in": np.eye(128, dtype=np.float16),
    }


def _dcn_core_kernel(tc, outs, ins):
    nc = tc.nc
    out_d = outs["out"]
    A = mybir.AluOpType

    with ExitStack() as ctx:
        consts = ctx.enter_context(tc.tile_pool(name="consts", bufs=1))
        idxp = ctx.enter_context(tc.tile_pool(name="idx", bufs=1))
        gath = ctx.enter_context(tc.tile_pool(name="gath", bufs=5))
        pmp = ctx.enter_context(tc.tile_pool(name="pm", bufs=2))
        wrp = ctx.enter_context(tc.tile_pool(name="wrp", bufs=1))
        colp = ctx.enter_context(tc.tile_pool(name="col", bufs=2))
        outp = ctx.enter_context(tc.tile_pool(name="outsb", bufs=2))
        psums = ctx.enter_context(tc.tile_pool(name="psums", bufs=4, space="PSUM"))
        psumc = ctx.enter_context(tc.tile_pool(name="psumc", bufs=1, space="PSUM"))

        pos_idx = consts.tile([128, CLQ, 64, 2], F32)
        pos_w4 = consts.tile([128, K, 2, NB], F32)
        # first-call slice lands first so the index math (and the first
        # gather) starts as early as possible
        nc.sync.dma_start(out=pos_idx[:, 0:1], in_=ins["pos_idx"][:, 0:1])
        nc.sync.dma_start(out=pos_idx[:, 1:], in_=ins["pos_idx"][:, 1:])
        nc.scalar.dma_start(out=pos_w4, in_=ins["pos_w4"])
        # one strided DMA: dram [K, C, O] -> sbuf [c-part, K, O]
        wts = consts.tile([128, K, O], F16)
        wsrc = ins["wts"]
        w_ap = bass.AP(tensor=wsrc.tensor, offset=0,
                       ap=[[O, 128], [C * O, K], [1, O]])
        nc.scalar.dma_start(out=wts, in_=w_ap)
        bias_sb = consts.tile([128, 1], F32)
        nc.sync.dma_start(out=bias_sb, in_=ins["bias_in"])
        ident = consts.tile([128, 128], F16)
        nc.sync.dma_start(out=ident, in_=ins["ident_in"])
        nc.gpsimd.load_library(library_config.mlp)

        # ---- gather indices, computed straight into SWDGE wrap-16 layout.
        # Two passes: cl=0 (first 4 calls) first so gathers start early.
        xview = AP(tensor=ins["xcl"].tensor, offset=0,
                   ap=[[GSTEP, NV], [1, GELEM]])

        idxw_parts = []
        for pi, sl in enumerate((slice(0, 1), slice(1, CLQ))):
            n = sl.stop - sl.start
            cpos = idxp.tile([128, n, 64, 2], F32, name=f"cpos{pi}")
            fi = idxp.tile([128, n, 64, 2], I32, name=f"fi{pi}")
            ff = idxp.tile([128, n, 64, 2], F32, name=f"ff{pi}")
            gf = idxp.tile([128, n, 64], F32, name=f"gf{pi}")
            idxw_p = idxp.tile([128, n, 64], I16, name=f"idxw{pi}")
            nc.vector.tensor_scalar(out=cpos, in0=pos_idx[:, sl],
                                    scalar1=-0.5, scalar2=64.5,
                                    op0=A.max, op1=A.min)
            nc.vector.tensor_copy(out=fi, in_=cpos)
            nc.vector.tensor_copy(out=ff, in_=fi)
            nc.vector.scalar_tensor_tensor(out=gf, in0=ff[:, :, :, 0],
                                           scalar=64.0, in1=ff[:, :, :, 1],
                                           op0=A.mult, op1=A.add)
            nc.vector.tensor_copy(out=idxw_p, in_=gf)
            idxw_parts.append(idxw_p)

        def idxw_call(cl):
            return idxw_parts[0][:, 0, :] if cl == 0 else idxw_parts[1][:, cl - 1, :]

        # ---- bilinear corner weights (pixel-major; validity masks folded in)
        cw = idxp.tile([128, K, 2, NB], F32)
        nc.vector.tensor_scalar(out=cw, in0=pos_w4, scalar1=-0.5, scalar2=64.5,
                                op0=A.max, op1=A.min)
        fiw = idxp.tile([128, K, 2, NB], I32)
        nc.vector.tensor_copy(out=fiw, in_=cw)
        fw = idxp.tile([128, K, 2, NB], F32)
        nc.vector.tensor_copy(out=fw, in_=fiw)
        frac = idxp.tile([128, K, 2, NB], F32)
        nc.vector.scalar_tensor_tensor(out=frac, in0=cw, scalar=0.5, in1=fw,
                                       op0=A.add, op1=A.subtract)
        va = idxp.tile([128, K, 2, NB], F32)
        vb = idxp.tile([128, K, 2, NB], F32)
        nc.vector.tensor_scalar(out=va, in0=fw, scalar1=1.0, scalar2=None,
                                op0=A.is_ge)
        nc.vector.tensor_scalar(out=vb, in0=fw, scalar1=64.0, scalar2=None,
                                op0=A.is_le)
        nc.vector.tensor_tensor(out=va, in0=va, in1=vb, op=A.mult)
        nc.vector.tensor_scalar(out=vb, in0=fw, scalar1=63.0, scalar2=None,
                                op0=A.is_le)
        w0 = idxp.tile([128, K, 2, NB], F32)
        w1 = idxp.tile([128, K, 2, NB], F32)
        nc.vector.tensor_scalar(out=w0, in0=frac, scalar1=-1.0, scalar2=1.0,
                                op0=A.mult, op1=A.add)
        nc.vector.tensor_tensor(out=w0, in0=w0, in1=va, op=A.mult)
        nc.vector.tensor_tensor(out=w1, in0=frac, in1=vb, op=A.mult)
        w4 = idxp.tile([128, K, 4, NB], F16)
        wy = (w0, w1)
        wx = (w0, w1)
        for ci, (dy, dx) in enumerate(CORNERS):
            nc.vector.tensor_tensor(
                out=w4[:, :, ci, :], in0=wy[dy][:, :, 0, :], in1=wx[dx][:, :, 1, :],
                op=A.mult)

        qidx = 0
        for ch in range(CHUNKS):
            # one 1-bank PSUM tile per 512-column accumulation group —
            # chunk 2's group m only waits for chunk 1's group m to evacuate
            conv_ms = [psumc.tile([128, 512], F32, space="PSUM",
                                  name=f"convps{m}") for m in range(PXC // 512)]
            bs = ch * NBC
            for k in range(K):
                gk = gath.tile([128, NBC, GELEM], F16, name="gk")
                for s in range(2):
                    nc.gpsimd.dma_gather(
                        out_ap=gk[:, s * 8:(s + 1) * 8, :],
                        in_ap=xview,
                        idxs_ap=idxw_call(qidx // NQ),
                        num_idxs=1024,
                        num_idxs_reg=1024,
                        elem_size=GELEM,
                        elem_step=GSTEP,
                        queue_num=qidx % NQ,
                    )
                    qidx += 1
                # weighted-diagonal moving operands: Dk[q, ci, b, j] =
                # ident[q, j] * w4[q, k, ci, bs+b]; the corner SUM rides the
                # PE's fp32 PSUM accumulation.  For a third of the taps, ACT
                # materialises the broadcast weights so the DVE multiply is a
                # stride-1 fp16 tensor_tensor (2x DVE perf mode) — balances
                # the diag-build load between ACT and DVE.
                dk = pmp.tile([128, 4, NBC, C], F16)
                i0 = ident[:, :]
                ident_b = bass.AP(tensor=i0.tensor, offset=i0.offset,
                                  ap=[i0.ap[0], [0, 4], [0, NBC], [1, C]])
                wv = w4[:, k, :, bs:bs + NBC]
                w_b = bass.AP(tensor=wv.tensor, offset=wv.offset,
                              ap=[wv.ap[0], wv.ap[1], wv.ap[2], [0, C]])
                if (ch * K + k) % 3 == 0 and (ch, k) != (0, 0):
                    w4rep = wrp.tile([128, 4, NBC, C], F16)
                    nc.scalar.copy(out=w4rep, in_=w_b)
                    nc.vector.tensor_tensor(out=dk[:, :, :, :], in0=ident_b,
                                            in1=w4rep, op=A.mult)
                else:
                    nc.vector.tensor_tensor(out=dk[:, :, :, :], in0=ident_b,
                                            in1=w_b, op=A.mult)
                # per pixel block: psum[c, j] += sum_ci gk_ci.T @ diag(w_ci)
                colk = colp.tile([128, PXC], F16)
                for bg in range(NBC // 4):
                    pst = psums.tile([128, 512], F32, space="PSUM")
                    for j in range(4):
                        b = bg * 4 + j
                        for ci in range(4):
                            nc.tensor.matmul(
                                out=pst[:, j * 128:(j + 1) * 128],
                                lhsT=gk[:, b, ci * C:(ci + 1) * C],
                                rhs=dk[:, ci, b, :],
                                start=(ci == 0), stop=(ci == 3))
                    nc.scalar.copy(out=colk[:, bg * 512:(bg + 1) * 512], in_=pst)
                for m in range(PXC // 512):
                    nc.tensor.matmul(
                        out=conv_ms[m][:, :],
                        lhsT=wts[:, k, :],
                        rhs=colk[:, m * 512:(m + 1) * 512],
                        start=(k == 0), stop=(k == K - 1))
            # evacuate per 512-column accumulation group so the tail pipelines
            # with the final conv matmuls
            out_sb = outp.tile([128, PXC], F32)
            for m in range(PXC // 512):
                nc.scalar.activation(out=out_sb[:, m * 512:(m + 1) * 512],
                                     in_=conv_ms[m][:, :],
                                     func=mybir.ActivationFunctionType.Identity,
                                     bias=bias_sb[:, :], scale=1.0)
                nc.sync.dma_start(
                    out=out_d[:, ch * PXC + m * 512:ch * PXC + (m + 1) * 512],
                    in_=out_sb[:, m * 512:(m + 1) * 512])


_IN_SPECS = {
    "xcl": ((TOT_PX, 2 * C), np.float16),
    "pos_idx": ((128, CLQ, 64, 2), np.float32),
    "pos_w4": ((128, K, 2, NB), np.float32),
    "wts": ((K, C, O), np.float16),
    "bias_in": ((O, 1), np.float32),
    "ident_in": ((128, 128), np.float16),
}

_prog_cache = {}


def _build_program():
    if "nc" in _prog_cache:
        return _prog_cache["nc"]
    nc = bacc.Bacc("TRN2", target_bir_lowering=False, debug=False,
                   num_swdge_queues=NQ)
    ins = {}
    for name, (shape, dtype) in _IN_SPECS.items():
        ins[name] = nc.dram_tensor(
            name, list(shape), mybir.dt.from_np(np.dtype(dtype)),
            kind="ExternalInput").ap()
    outs = {"out": nc.dram_tensor("out", [O, HW], F32,
                                  kind="ExternalOutput").ap()}
    with tile.TileContext(nc) as tc:
        _dcn_core_kernel(tc, outs, ins)
    nc.compile()
    _prog_cache["nc"] = nc
    return nc


def run_dcn(x, offset, weight, bias, trace=False):
    x = np.ascontiguousarray(x, dtype=np.float32)
    offset = np.ascontiguousarray(offset, dtype=np.float32)
    weight = np.ascontiguousarray(weight, dtype=np.float32)
    bias = np.ascontiguousarray(bias, dtype=np.float32)
    B = x.shape[0]
    in_maps = [_prep_core_inputs(x[b], offset[b], weight, bias)
               for b in range(B)]
    nc = _build_program()
    res = run_bass_kernel_spmd(nc, in_maps, core_ids=list(range(B)), trace=trace)
    out = np.stack([r["out"] for r in res.results]).reshape(B, O, H, W)
    return out, res


def kernel(x, offset, weight, bias):
    out, _ = run_dcn(x, offset, weight, bias)
    return out.astype(np.float32)
